# revision 31
# baseline (speedup 1.0000x reference)
"""FNO block (nn_FNOBlock_48962627175213) as a Bass/Tile kernel on 8 trn2 cores.

Math: only 64 complex rfft modes (32 low + 32 high) survive into out_ft, so
rfft/irfft collapse into skinny DFT matmuls against precomputed bases.
Data-parallel over batch: each core takes 4 of the 32 batches (256 rows).

v3.1 design (vs v2):
  - phi (the data-dependent complex factor) folds into the XS-stage dtile
    operand instead of a post-spectral DVE rotation: the head builds
    per-(group, batch) diagonal tiles dt[t][j] = maskSD * phiD on DVE, off
    the critical chain.  Removes the rotation stage + one copy from the
    fwd->silu critical path.
  - DMAs on a queue serialize with each other AND that queue's engine ops
    (incl. the act-table load), so the Act queue carries only what fits
    before its silu stream: CA-early, table, Wq2/Wq3, x4_00, CA-late.
  - CA is split into early (phi/h/dtile consts) and late (head-B consts)
    halves; Wd0 loads as two half DMAs on SP and Pool; spec runs dout=1
    (Wq2/Wq3) before dout=0 (Wd0) to match arrival order.
  - fwd and inverse DFT run as fp8e4 DoubleRow matmuls.  Spectral branch
    is ~1e-4 of the output so fp8 there is numerically free.
  - scale folding: F x64, dtile /128, W x4096, G x8 => spectral PSUM at
    2^14 x true; time branch matmuls at 2^14 via scaled lin_w; the final
    activation applies scale=2^-14 and the true-scale bias.
  - out tiles [512,1536,2048,1536,2048,512] per group alternate a 4-bank
    and a 3-bank PSUM pool (+1 bank mid ring).
"""
import sys

if '/opt/trn_rl_repo' not in sys.path:
    sys.path.insert(0, '/opt/trn_rl_repo')

import numpy as np
import ml_dtypes

import concourse.bass as bass
import concourse.mybir as mybir
from concourse.tile import TileContext
from concourse.bass_utils import run_bass_kernel_spmd

FP = mybir.dt.float32
BF = mybir.dt.float16
F8 = mybir.dt.float8e4
E4 = ml_dtypes.float8_e4m3
DR = mybir.MatmulPerfMode.DoubleRow
AF = mybir.ActivationFunctionType

B, C, L, M, EMB, HID = 32, 64, 8192, 32, 256, 64
K = L // 2 + 1
NEG0 = K - M          # 4065
N_CORES = 8
B_LOC = B // N_CORES  # 4
ROWS = B_LOC * C      # 256

SF = 64.0         # F basis scale (fp8)
SD = 1.0 / 128.0  # dtile (phi) scale
SW = 4096.0       # spectral weight scale (fp8)
SG = 8.0          # inverse basis scale (fp8)
ST = 16384.0      # time-branch weight scale == SF*SD*SW*SG (2^14)
DESCALE = 1.0 / ST

# out-tile column sizes per row-group (sum 8192); alternate PSUM pools A/B
SZ = [512, 1536, 2048, 1536, 2048, 512]
OFF = [0, 512, 2048, 4096, 5632, 7680]
NT = len(SZ)

# CA packed-const column offsets.  Early block (0:CA_E) carries everything
# the phi/dtile/h chain needs; late block has the head-B consts.
CA_EMB = 0      # embT (per-core) [128, 8]
CA_A3 = 8       # phi projector [128, 512]
CA_W1 = 520     # w1T [128, 128]
CA_B1 = 648     # b1 [64, 1]
CA_ID4 = 649    # eye(4) [4, 4]
CA_MSK = 653    # maskSD [128, 32]
CA_E = 685      # end of early block
CA_LW = 685     # lin_w.T * ST tiled [128, 64]
CA_W2 = 749     # w2T [64, 128]
CA_B2 = 877     # b2 tiled [4, 128]
CA_LB = 1005    # lin_b tiled [4, 64]
CA_SEL = 1069   # batch selector [5, 256]
CA_BSEL = 1325  # bias selector [4, 4]
CA_ID64 = 1329  # eye(64) [64, 64]
CA_COLS = 1393


# --------------------------------------------------------------------------
# host-side constant builders
# --------------------------------------------------------------------------
def _build_constants(weights_pos, weights_neg, A_real_pos, A_imag_pos,
                     A_real_neg, A_imag_neg, tm_w1, tm_b1, tm_w2, tm_b2,
                     lin_w, lin_b):
    n = np.arange(L, dtype=np.float64)
    s = 1.0 / np.sqrt(L)

    # fwd DFT basis [8192, 128], col = br*64 + m (cos) / br*64+32+m (-sin)
    F = np.zeros((L, 128), np.float64)
    for br in range(2):
        for m in range(M):
            k = m if br == 0 else NEG0 + m
            ang = 2.0 * np.pi * k * n / L
            F[:, br * 64 + m] = np.cos(ang) * s
            F[:, br * 64 + 32 + m] = -np.sin(ang) * s
    # DoubleRow layout [128 p, 2 j, 32 c, 128 mode]: F_dr[p,j,c,m]=F[(2c+j)*128+p, m]
    F_dr = (F * SF).reshape(32, 2, 128, 128).transpose(2, 1, 0, 3)
    F_dr = np.ascontiguousarray(F_dr).astype(E4)

    # inverse basis [128, 8192], row = d*64 + br*32 + m (pocketfft irfft
    # semantics: Im parts of DC and Nyquist are discarded)
    G = np.zeros((128, L), np.float64)
    for br in range(2):
        for m in range(M):
            k = m if br == 0 else NEG0 + m
            ang = 2.0 * np.pi * k * n / L
            if k == 0:
                G[br * 32 + m] = s
            elif k == L // 2:
                G[br * 32 + m] = np.cos(np.pi * n) * s
            else:
                G[br * 32 + m] = 2.0 * np.cos(ang) * s
                G[64 + br * 32 + m] = -2.0 * np.sin(ang) * s
    # DoubleRow layout [64 p=(br,m), 2 j=d, 8192]
    G_dr = (G * SG).reshape(2, 64, L).transpose(1, 0, 2)
    G_dr = np.ascontiguousarray(G_dr).astype(E4)

    # spectral weights split by output half so spectral matmuls land at
    # partition base 0: Wd[dout] [128 rows=(din,i), (br*32+m)*64 + o];
    # dout=0 -> [wr; -wi], dout=1 -> [wi; wr]
    Wd = np.zeros((2, 128, 4096), np.float32)
    for br, wfull in ((0, weights_pos), (1, weights_neg)):
        for m in range(M):
            wr = wfull[:, :, m, 0]
            wi = wfull[:, :, m, 1]
            c = (br * 32 + m) * 64
            Wd[0, 0:64, c:c + 64] = wr
            Wd[0, 64:128, c:c + 64] = -wi
            Wd[1, 0:64, c:c + 64] = wi
            Wd[1, 64:128, c:c + 64] = wr
    Wd = (Wd * SW).astype(E4)

    # phi projector with signs baked for the rotated-XS build:
    # Astack3 [EMB, 256], col = dout*128 + br*64 + dd*32 + m:
    #   dout=0: dd=0 -> A_real[br], dd=1 -> -A_imag[br]   (re' row coeffs)
    #   dout=1: dd=0 -> A_imag[br], dd=1 ->  A_real[br]   (im' row coeffs)
    # (rtp rows: dd=0 cos=re plane, dd=1 -sin=im plane)
    A3 = np.zeros((EMB, 256), np.float32)
    for br, Ar, Ai in ((0, A_real_pos, A_imag_pos), (1, A_real_neg, A_imag_neg)):
        A3[:, 0 * 128 + br * 64 + 0:0 * 128 + br * 64 + 32] = Ar.T
        A3[:, 0 * 128 + br * 64 + 32:0 * 128 + br * 64 + 64] = -Ai.T
        A3[:, 1 * 128 + br * 64 + 0:1 * 128 + br * 64 + 32] = Ai.T
        A3[:, 1 * 128 + br * 64 + 32:1 * 128 + br * 64 + 64] = Ar.T
    # k-chunk repack (SBUF tiles cap at 128 partitions): [128, 2*256]
    A3 = np.ascontiguousarray(
        A3.reshape(2, 128, 256).transpose(1, 0, 2).reshape(128, 512))

    w1T = tm_w1.T.astype(np.float32)  # [256, 64] -> [128, 2*64]
    w1T = np.ascontiguousarray(
        w1T.reshape(2, 128, 64).transpose(1, 0, 2).reshape(128, 128))

    # batch selector for gamma broadcast: selt[p, t*128 + j*64 + c] = (p==2t+j)
    selt = np.zeros((4, 256), np.float32)
    for t in range(2):
        for j in range(2):
            selt[2 * t + j, t * 128 + j * 64:t * 128 + (j + 1) * 64] = 1.0
    # bias selector: cols j*2+t pick batch 2t+j
    bsel = np.zeros((4, 4), np.float32)
    for j in range(2):
        for t in range(2):
            bsel[2 * t + j, j * 2 + t] = 1.0

    CA = np.zeros((128, CA_COLS), np.float32)
    CA[:, CA_A3:CA_A3 + 512] = A3
    CA[:, CA_W1:CA_W1 + 128] = w1T
    CA[0:64, CA_B1] = tm_b1
    CA[0:4, CA_ID4:CA_ID4 + 4] = np.eye(4)
    CA[:, CA_MSK:CA_MSK + 32] = np.tile(np.eye(32) * SD, (4, 1))
    CA[:, CA_LW:CA_LW + 64] = np.tile(lin_w.T * ST, (2, 1))
    CA[0:64, CA_W2:CA_W2 + 128] = tm_w2.T
    CA[0:4, CA_B2:CA_B2 + 128] = np.tile(tm_b2, (4, 1))
    CA[0:4, CA_LB:CA_LB + 64] = np.tile(lin_b, (4, 1))
    CA[0:4, CA_SEL:CA_SEL + 256] = selt
    CA[4, CA_SEL:CA_SEL + 256] = 1.0  # ones row: selector matmul yields 1+gamma
    CA[0:4, CA_BSEL:CA_BSEL + 4] = bsel
    CA[0:64, CA_ID64:CA_ID64 + 64] = np.eye(64)
    out = dict(
        F0=np.ascontiguousarray(F_dr[:, :, 0:16, :]),
        F1=np.ascontiguousarray(F_dr[:, :, 16:32, :]),
        CA=CA.astype(np.float16),
        G=np.ascontiguousarray(G_dr),
    )
    out["Wd0"] = np.ascontiguousarray(Wd[0])
    out["Wq2"] = np.ascontiguousarray(Wd[1][:, 0:2048])
    out["Wq3"] = np.ascontiguousarray(Wd[1][:, 2048:4096])
    return out


def _stage_x(x_loc):
    """per-core x staging: fp16 row-major + fp8 DoubleRow-transposed."""
    xf = x_loc.reshape(ROWS, L).astype(np.float32)
    x16 = np.ascontiguousarray(xf, np.float16)
    # xT_dr[p, j, c, row] = x[row, (2c+j)*128 + p], split by row-group
    xT = xf.T.reshape(32, 2, 128, ROWS).transpose(2, 1, 0, 3)  # [128,2,32,256]
    out = {"x4": x16}
    for t in range(2):
        rows = slice(t * 128, (t + 1) * 128)
        out[f"xT{t}0"] = np.ascontiguousarray(xT[:, :, 0:16, rows]).astype(E4)
        out[f"xT{t}1"] = np.ascontiguousarray(xT[:, :, 16:32, rows]).astype(E4)
    return out


# --------------------------------------------------------------------------
# walrus workaround: this container's walrus rejects >1 sync-wait on
# TPB_CTRL lowering (Drain/NoOp). Split extra waits onto preceding NOPs.
# --------------------------------------------------------------------------
def _split_multiwait(nc, max_waits=1):
    for f in nc.m.functions:
        for blk in f.blocks:
            new = []
            changed = False
            for inst in blk.instructions:
                si = inst.sync_info
                if (si is not None and len(si.on_wait) > max_waits):
                    waits = list(si.on_wait)
                    head, tail = waits[:-max_waits], waits[-max_waits:]
                    for j, w in enumerate(head):
                        nop = mybir.InstNoOp(name=f"{inst.name}-ws{j}",
                                             ins=[], outs=[])
                        nop.engine = inst.engine
                        nop.sync_info = mybir.SyncInfo(on_wait=[w], on_update=[])
                        new.append(nop)
                    inst.sync_info = mybir.SyncInfo(on_wait=tail,
                                                    on_update=list(si.on_update))
                    changed = True
                new.append(inst)
            if changed:
                blk.instructions = new


# --------------------------------------------------------------------------
# the bass program (input-value independent; built once)
# --------------------------------------------------------------------------
def _build_nc(split=True):
    nc = bass.Bass("TRN2")
    d = {}
    specs = [
        ("x4", [ROWS, L], BF),
        ("xT00", [128, 2, 16, 128], F8), ("xT01", [128, 2, 16, 128], F8),
        ("xT10", [128, 2, 16, 128], F8), ("xT11", [128, 2, 16, 128], F8),
        ("F0", [128, 2, 16, 128], F8), ("F1", [128, 2, 16, 128], F8),
        ("CA", [128, CA_COLS], BF),
        ("G", [64, 2, 8192], F8),
        ("Wd0", [128, 4096], F8),
        ("Wq2", [128, 2048], F8),
        ("Wq3", [128, 2048], F8),
    ]
    for name, shape, dt_ in specs:
        d[name] = nc.dram_tensor(name, shape, dt_, kind="ExternalInput")
    y = nc.dram_tensor("y", [ROWS, L], BF, kind="ExternalOutput")

    with TileContext(nc) as tc:
        from contextlib import ExitStack
        with ExitStack() as ctx:
            const = ctx.enter_context(tc.tile_pool(name="const", bufs=1))
            small = ctx.enter_context(tc.tile_pool(name="small", bufs=1))
            sop = ctx.enter_context(tc.tile_pool(name="sop", bufs=8))

            # ---- SBUF const tiles ----
            ca = const.tile([128, CA_COLS], BF, tag="CA", name="CA")
            x4t = [[const.tile([128, SZ[k]], BF, tag=f"x4_{t}{k}",
                               name=f"x4_{t}{k}") for k in range(NT)]
                   for t in range(2)]
            Fh = [const.tile([128, 2, 16, 128], F8, tag=f"F{h}", name=f"F{h}")
                  for h in range(2)]
            xTg = [[const.tile([128, 2, 16, 128], F8, tag=f"xT{t}{h}",
                               name=f"xT{t}{h}") for h in range(2)]
                   for t in range(2)]
            Gh = [const.tile([64, 2, 4096], F8, tag=f"G{h}", name=f"G{h}")
                  for h in range(2)]
            Wd0 = const.tile([128, 4096], F8, tag="Wd0", name="Wd0")
            Wq2 = const.tile([128, 2048], F8, tag="Wq2", name="Wq2")
            Wq3 = const.tile([128, 2048], F8, tag="Wq3", name="Wq3")

            def x4load(eng, t, k):
                eng.dma_start(
                    out=x4t[t][k][:],
                    in_=d["x4"][t * 128:(t + 1) * 128,
                                OFF[k]:OFF[k] + SZ[k]])

            def x4rhs(t, k, i):
                return x4t[t][k][:, i * 512:(i + 1) * 512]

            def gload(eng, h, q):
                # quarter [64, 2, 1024] DMAs so arrival tracks per-tile need
                c0 = q * 1024
                eng.dma_start(
                    out=Gh[h][:, :, c0:c0 + 1024],
                    in_=d["G"][:, :, h * 4096 + c0:h * 4096 + c0 + 1024])

            # ---- load schedule phase 1 (critical path) ----
            # Act: CA-early, act-table, Wq2, Wq3, x4_00, CA-late
            scr = small.tile([1, 1], FP, tag="scr", name="scr")
            nc.vector.memset(scr[:], 0.0)
            dum = small.tile([1, 1], FP, tag="dum", name="dum")
            nc.scalar.dma_start(out=ca[:], in_=d["CA"][:])
            nc.scalar.activation(dum[:], scr[:], AF.Silu)
            nc.scalar.dma_start(out=Wq2[:], in_=d["Wq2"][:])

            # SP: F0, F1, Wd0-half0, G0j0 (phase 2: x4 tiles)
            nc.sync.dma_start(out=Fh[0][:], in_=d["F0"][:])
            nc.sync.dma_start(out=Fh[1][:], in_=d["F1"][:])
            nc.sync.dma_start(out=Wd0[:, 0:2048], in_=d["Wd0"][:, 0:2048])
            gload(nc.sync, 0, 0)

            # Pool: xT00, xT01, Wd0-half1, G0j1 (phase 2: x4_01, G1, xT1)
            nc.gpsimd.dma_start(out=xTg[0][0][:], in_=d["xT00"][:])
            nc.gpsimd.dma_start(out=xTg[0][1][:], in_=d["xT01"][:])
            nc.gpsimd.dma_start(out=Wd0[:, 2048:4096],
                                in_=d["Wd0"][:, 2048:4096])
            nc.gpsimd.dma_start(out=Wq3[:], in_=d["Wq3"][:])
            gload(nc.gpsimd, 0, 1)

            # ---- head A: phi projector, h (MLP layer 1), dtile build ----
            phiT3_sb = small.tile([B_LOC, 256], BF, tag="phiT3")
            phiD = small.tile([128, 8], FP, tag="phiD")
            h_sb = small.tile([HID, B_LOC], BF, tag="h_sb")
            gbT_sb = small.tile([5, 128], BF, tag="gbT")
            biasvec = small.tile([4, 64], BF, tag="biasvec")
            bt_sb = small.tile([128, 2], FP, tag="bt_sb")
            linwb2 = [small.tile([128, 128], BF, tag=f"lw{t}", name=f"lw{t}")
                      for t in range(2)]
            dt_sb = [[small.tile([128, 64], BF, tag=f"dt{t}{j}",
                                 name=f"dt{t}{j}") for j in range(2)]
                     for t in range(2)]
            tmp44 = small.tile([4, 64], BF, tag="tmp44")
            nc.vector.memset(gbT_sb[:], 1.0)  # row 4 stays 1 (1+gamma)
            for t in range(2):
                nc.vector.memset(linwb2[t][:], 0.0)

            pm = tc.alloc_tile_pool(name="ps_mid", bufs=1, space="PSUM")
            ph = tc.alloc_tile_pool(name="ps_head", bufs=2, space="PSUM")
            h_p = ph.tile([HID, B_LOC], FP, tag="hps", name="h_p")
            for kc in range(2):
                nc.tensor.matmul(h_p[:],
                                 lhsT=ca[:, CA_W1 + kc * 64:CA_W1 + (kc + 1) * 64],
                                 rhs=ca[:, kc * 4:(kc + 1) * 4],
                                 start=(kc == 0), stop=(kc == 1))
            phiT3_p = ph.tile([B_LOC, 256], FP, tag="hps", name="phiT3_p")
            for kc in range(2):
                nc.tensor.matmul(phiT3_p[:],
                                 lhsT=ca[:, kc * 4:(kc + 1) * 4],
                                 rhs=ca[:, CA_A3 + kc * 256:CA_A3 + (kc + 1) * 256],
                                 start=(kc == 0), stop=(kc == 1))
            nc.scalar.activation(h_sb[:], h_p[:], AF.Silu,
                                 bias=ca[0:64, CA_B1:CA_B1 + 1])
            x4load(nc.scalar, 0, 0)
            x4load(nc.scalar, 0, 1)
            gload(nc.scalar, 0, 3)
            gload(nc.scalar, 1, 1)
            gload(nc.scalar, 1, 3)
            nc.vector.tensor_copy(phiT3_sb[:], phiT3_p[:])

            # phiD[(br,dd,m), dout*4 + b] = signed phi coefficient
            prp3 = ph.tile([128, 8], BF, tag="hps", name="prp3")
            for dout in range(2):
                nc.tensor.transpose(prp3[:, dout * 4:(dout + 1) * 4],
                                    phiT3_sb[:, dout * 128:(dout + 1) * 128],
                                    ca[0:4, CA_ID4:CA_ID4 + 4])
            nc.vector.tensor_copy(phiD[:], prp3[:])
            # dtile build: dt[t][j][:, dout*32:(dout+1)*32] = maskSD * phiD col
            for t in range(2):
                for j in range(2):
                    for dout in range(2):
                        col = dout * 4 + 2 * t + j
                        nc.vector.tensor_scalar_mul(
                            dt_sb[t][j][:, dout * 32:(dout + 1) * 32],
                            ca[:, CA_MSK:CA_MSK + 32],
                            phiD[:, col:col + 1])

            # ---- mid-pipeline state ----
            RT_sb = [small.tile([128, 128], BF, tag=f"RT{t}", name=f"RT{t}")
                     for t in range(2)]
            XS_sb = [small.tile([128, 128], BF, tag=f"XS{t}", name=f"XS{t}")
                     for t in range(2)]
            spec_sb = [small.tile([64, 256], BF, tag=f"spec{t}",
                                  name=f"spec{t}") for t in range(2)]
            R2f = [small.tile([64, 2, 128], F8, tag=f"R2f{t}", name=f"R2f{t}")
                   for t in range(2)]

            rtp_t = [None, None]

            def fwd_half(t, hh):
                # fwd DFT: 16 DoubleRow matmuls per half, K=256 per matmul
                if hh == 0:
                    rtp_t[t] = pm.tile([128, 128], FP, tag="mid",
                                       name=f"rtp{t}")
                rtp = rtp_t[t]
                for cc in range(16):
                    c = hh * 16 + cc
                    nc.tensor.matmul(rtp[:],
                                     lhsT=Fh[hh][:, :, cc, :],
                                     rhs=xTg[t][hh][:, :, cc, :],
                                     start=(c == 0), stop=(c == 31),
                                     perf_mode=DR)
                if hh == 1:
                    # copy in j-halves so the j=0 XS matmuls start sooner
                    nc.vector.tensor_copy(RT_sb[t][:, 0:64], rtp[:, 0:64])
                    nc.vector.tensor_copy(RT_sb[t][:, 64:128],
                                          rtp[:, 64:128])

            def fwd_xs(t):

                # XS: transpose+phi via per-(br,j,dout) dtile matmuls
                for br in range(2):
                    xsp = pm.tile([128, 64], FP, tag="mid", name=f"xsp{t}{br}")
                    psl = slice(br * 64, br * 64 + 64)
                    for j in range(2):
                        for dout in range(2):
                            nc.tensor.matmul(
                                xsp[dout * 64:(dout + 1) * 64, j::2],
                                lhsT=RT_sb[t][psl, j * 64:(j + 1) * 64],
                                rhs=dt_sb[t][j][psl,
                                                dout * 32:(dout + 1) * 32],
                                start=True, stop=True)
                    nc.vector.tensor_copy(XS_sb[t][:, br * 64:(br + 1) * 64],
                                          xsp[:])

            def spec_mm(t):
                # spectral: per-(mode, dout) matmuls, N=2, all base-0.
                # br-major: br0's weights (Wq2, Wd0-half0) arrive first.
                spp = pm.tile([64, 256], FP, tag="mid", name=f"spp{t}")
                for br in range(2):
                    for dout in (1, 0):
                        for m in range(M):
                            if dout == 0:
                                wsl = Wd0[:, br * 2048 + m * 64:
                                          br * 2048 + (m + 1) * 64]
                            else:
                                wt = Wq2 if br == 0 else Wq3
                                wsl = wt[:, m * 64:(m + 1) * 64]
                            col = dout * 128 + (br * 32 + m) * 2
                            nc.tensor.matmul(
                                spp[0:64, col:col + 2],
                                lhsT=wsl,
                                rhs=XS_sb[t][:, br * 64 + m * 2:
                                             br * 64 + (m + 1) * 2],
                                start=True, stop=True)
                nc.vector.tensor_copy(spec_sb[t][:], spp[:])

            def spec_r2(t):
                # R2 transposes -> r2p [64 p=(br,m), (dout, j, o)] == R2f
                r2p = pm.tile([64, 256], FP, tag="mid", name=f"r2p{t}")
                for dout in range(2):
                    for j in range(2):
                        nc.tensor.matmul(
                            r2p[0:64, dout * 128 + j * 64:
                                dout * 128 + (j + 1) * 64],
                            lhsT=spec_sb[t][0:64,
                                            dout * 128 + j:dout * 128 + 128:2],
                            rhs=ca[0:64, CA_ID64:CA_ID64 + 64],
                            start=True, stop=True)
                nc.vector.tensor_copy(R2f[t][:], r2p[:])

            fwd_half(0, 0)
            fwd_half(0, 1)
            fwd_xs(0)

            # ---- load schedule phase 2 ----
            # SP: x4_02, G0q2, x4_03, G1q0, x4_05, G1q2, x4_04
            x4load(nc.sync, 0, 2)
            gload(nc.sync, 0, 2)
            x4load(nc.sync, 0, 3)
            gload(nc.sync, 1, 0)
            x4load(nc.sync, 0, 5)
            gload(nc.sync, 1, 2)
            x4load(nc.sync, 0, 4)
            # Pool: xT10, xT11
            nc.gpsimd.dma_start(out=xTg[1][0][:], in_=d["xT10"][:])
            nc.gpsimd.dma_start(out=xTg[1][1][:], in_=d["xT11"][:])

            # ---- head B: gbT, bias vector, scaled time weights ----
            gbT_p = ph.tile([4, 128], FP, tag="hps", name="gbT_p")
            nc.tensor.matmul(gbT_p[:], lhsT=h_sb[:],
                             rhs=ca[0:64, CA_W2:CA_W2 + 128],
                             start=True, stop=True)
            nc.vector.tensor_add(gbT_sb[0:4, :], gbT_p[:],
                                 ca[0:4, CA_B2:CA_B2 + 128])
            rep_p = [ph.tile([128, 64], FP, tag="hps", name=f"rep{t}")
                     for t in range(2)]
            for t in range(2):
                nc.tensor.matmul(rep_p[t][:],
                                 lhsT=ca[0:5, CA_SEL + t * 128:
                                         CA_SEL + (t + 1) * 128],
                                 rhs=gbT_sb[0:5, 0:64], start=True, stop=True)
            # biasvec = gamma*lin_b + lin_b + beta (true scale)
            nc.vector.tensor_mul(tmp44[:], gbT_sb[0:4, 0:64],
                                 ca[0:4, CA_LB:CA_LB + 64])
            nc.vector.tensor_add(tmp44[:], tmp44[:], ca[0:4, CA_LB:CA_LB + 64])
            nc.vector.tensor_add(biasvec[:], tmp44[:], gbT_sb[0:4, 64:128])
            # bt_sb[(j,o), t] = biasvec[2t+j, o] via 2 selector matmuls
            btp = ph.tile([128, 2], FP, tag="hps", name="btp")
            for j in range(2):
                nc.tensor.matmul(btp[j * 64:(j + 1) * 64, :],
                                 lhsT=biasvec[:],
                                 rhs=ca[0:4, CA_BSEL + j * 2:
                                        CA_BSEL + (j + 1) * 2],
                                 start=True, stop=True)
            nc.vector.tensor_copy(bt_sb[:], btp[:])
            # linwb2[t][(j,c),(j,o)] block-diag = lin_w.T*ST*(1+gamma[2t+j])
            for t in range(2):
                for j in range(2):
                    sl = slice(j * 64, (j + 1) * 64)
                    nc.vector.tensor_mul(linwb2[t][sl, sl],
                                         ca[sl, CA_LW:CA_LW + 64],
                                         rep_p[t][sl, :])

            spec_mm(0)
            spec_r2(0)
            ph.release()

            # ---- load schedule phase 3: x4 group 1 ----
            x4load(nc.sync, 1, 0)
            x4load(nc.sync, 1, 1)
            x4load(nc.sync, 1, 2)
            x4load(nc.gpsimd, 1, 3)
            x4load(nc.gpsimd, 1, 4)
            x4load(nc.gpsimd, 1, 5)
            # (G halves were loaded in phase 1 on SP/Pool)

            poA = tc.alloc_tile_pool(name="ps_oa", bufs=1, space="PSUM")
            poB = tc.alloc_tile_pool(name="ps_ob", bufs=1, space="PSUM")

            def out_tile(t, k):
                po = poA if k % 2 == 0 else poB
                sz = SZ[k]
                nch = sz // 512
                pos = po.tile([128, sz], FP, tag="po", name=f"po{t}{k}")
                for i in range(nch):
                    nc.tensor.matmul(
                        pos[:, i * 512:(i + 1) * 512],
                        lhsT=linwb2[t][:],
                        rhs=x4rhs(t, k, i),
                        start=True, stop=False)
                gh = 0 if k < 3 else 1
                gof = OFF[k] - gh * 4096
                for i in range(nch):
                    nc.tensor.matmul(
                        pos[:, i * 512:(i + 1) * 512],
                        lhsT=R2f[t][:],
                        rhs=Gh[gh][:, :, gof + i * 512:gof + (i + 1) * 512],
                        start=False, stop=True, perf_mode=DR)
                so = sop.tile([128, sz], BF, tag="so")
                nc.scalar.activation(so[:], pos[:], AF.Silu,
                                     bias=bt_sb[:, t:t + 1], scale=DESCALE)
                if (t, k) == (1, 4):
                    # split the late big store across both queues
                    for hf, eng in ((0, nc.gpsimd), (1, nc.sync)):
                        eng.dma_start(
                            out=y[t * 128:(t + 1) * 128,
                                  OFF[k] + hf * 1024:OFF[k] + (hf + 1) * 1024],
                            in_=so[:, hf * 1024:(hf + 1) * 1024])
                elif (t, k) == (1, 5):
                    nc.scalar.dma_start(
                        out=y[t * 128:(t + 1) * 128, OFF[k]:OFF[k] + sz],
                        in_=so[:])
                else:
                    eng = nc.gpsimd if (t * NT + k) % 2 == 0 else nc.sync
                    eng.dma_start(
                        out=y[t * 128:(t + 1) * 128, OFF[k]:OFF[k] + sz],
                        in_=so[:])

            out_tile(0, 0)
            out_tile(0, 1)
            fwd_half(1, 0)
            out_tile(0, 2)
            fwd_half(1, 1)
            out_tile(0, 3)
            fwd_xs(1)
            spec_mm(1)
            out_tile(0, 4)
            spec_r2(1)
            out_tile(0, 5)
            for k in range(NT):
                out_tile(1, k)
            poB.release()
            poA.release()
            pm.release()

    if split:
        _split_multiwait(nc)
    return nc


_NC = None


def _get_nc():
    global _NC
    if _NC is None:
        _NC = _build_nc()
    return _NC


def _core_inputs(x, emb, consts, core):
    b0 = core * B_LOC
    m = dict(consts)
    m.update(_stage_x(np.ascontiguousarray(x[b0:b0 + B_LOC])))
    eT = emb[b0:b0 + B_LOC].T.astype(np.float32)
    CA = consts["CA"].copy()
    CA[:, 0:8] = eT.reshape(2, 128, B_LOC).transpose(1, 0, 2).reshape(
        128, 8).astype(np.float16)
    m["CA"] = CA
    return m


def kernel(**inputs):
    inputs = {k: np.asarray(v) for k, v in inputs.items()}
    x, emb = inputs["x"], inputs["emb"]
    consts = _build_constants(**{k: v for k, v in inputs.items()
                                 if k not in ("x", "emb")})
    nc = _get_nc()

    in_maps = [_core_inputs(x, emb, consts, core) for core in range(N_CORES)]
    res = run_bass_kernel_spmd(nc, in_maps, core_ids=list(range(N_CORES)))
    out = np.empty((B, C, L), np.float32)
    for core in range(N_CORES):
        b0 = core * B_LOC
        out[b0:b0 + B_LOC] = res.results[core]["y"].astype(
            np.float32).reshape(B_LOC, C, L)
    return out
